# revision 1
# baseline (speedup 1.0000x reference)
"""Trainium2 Bass kernel for nn_CrossAttentionFusion (V=3, B=8192, H=2048, NH=16).

Strategy:
  - Data-parallel: batch B=8192 split across 8 NeuronCores (Bc=1024 each).
  - Feature-major activations on device: every tensor is [H, Bc] so all
    projections are PE matmuls (lhsT = W^T tile [128h x 128g], moving = act
    [128h x 512b]) with no on-device transposes.  Host transposes views and
    weights once (pure layout, no math).
  - fp32r matmuls (TF32-class rounding, 1 cyc/row at N=512 -> ~238ns/MM).
  - Softmax over V-1=2 key views collapses to a sigmoid:
        a0 = sigmoid((qh . (kh0 - kh1)) / sqrt(HD))
        ctx = v2_1 + a0 * (v2_0 - v2_1)
    so the k-side inner projection only needs Wik @ (k[s0] - k[s1]).
  - 27 HxH matmul-equivalents per core, PE-bound.
"""

import math

import numpy as np

V = 3
B = 8192
H = 2048
NH = 16
HD = H // NH
EPS = 1e-5
N_CORES = 8
BC = B // N_CORES          # 1024 batch columns per core
NT = H // 128              # 16 h-tiles
HALF = 512                 # matmul moving free dim
SCALE = 1.0 / math.sqrt(HD)

# others[i] = sources of keys/values for query view i
S0 = [1, 0, 0]
S1 = [2, 2, 1]

_CACHE = {}



def _build_program():
    import concourse.bass as bass
    import concourse.bacc as bacc
    import concourse.tile as tile
    import concourse.mybir as mybir

    f32 = mybir.dt.float32
    f32r = mybir.dt.float32r
    AF = mybir.ActivationFunctionType
    ALU = mybir.AluOpType

    nc = bacc.Bacc("TRN2", target_bir_lowering=False, debug=False,
                   num_devices=N_CORES)

    # ---- External I/O ----
    xT = nc.dram_tensor("xT", [V, H, BC], f32r, kind="ExternalInput").ap()
    wq = nc.dram_tensor("wq", [V, H, H], f32r, kind="ExternalInput").ap()
    wk = nc.dram_tensor("wk", [V, H, H], f32r, kind="ExternalInput").ap()
    wv = nc.dram_tensor("wv", [V, H, H], f32r, kind="ExternalInput").ap()
    wiq = nc.dram_tensor("wiq", [V, H, H], f32r, kind="ExternalInput").ap()
    wik = nc.dram_tensor("wik", [V, H, H], f32r, kind="ExternalInput").ap()
    wiv = nc.dram_tensor("wiv", [V, H, H], f32r, kind="ExternalInput").ap()
    wo = nc.dram_tensor("wo", [V, H, H], f32r, kind="ExternalInput").ap()
    wout = nc.dram_tensor("wout", [V, H, H], f32r, kind="ExternalInput").ap()
    # bias pack: [21,128,16]: bq(0-2) bk(3-5) bv(6-8) biq(9-11) biv(12-14)
    # bo(15-17) bout(18) gamma(19) beta(20); [:, :, gt] is per-partition
    bpk = nc.dram_tensor("bpk", [21, 128, NT], f32, kind="ExternalInput").ap()
    onesc = nc.dram_tensor("onesc", [128, 128], f32r, kind="ExternalInput").ap()
    out = nc.dram_tensor("out", [H, BC], f32, kind="ExternalOutput").ap()

    # ---- DRAM scratch ----
    def scr(name):
        return nc.dram_tensor(name, [V, H, BC], f32r).ap()

    k_s, v_s = scr("k_s"), scr("v_s")
    q2_s, dk2_s = scr("q2_s"), scr("dk2_s")
    dv2_s, v21_s = scr("dv2_s"), scr("v21_s")
    xacc = nc.dram_tensor("xacc", [H, BC], f32r).ap()
    a0_d = nc.dram_tensor("a0_d", [V, NT, BC], f32r).ap()

    with tile.TileContext(nc) as tc:
        ctxs = []

        def pool(name, bufs):
            p = tc.tile_pool(name=name, bufs=bufs)
            ctxs.append(p)
            return p.__enter__()

        xin = pool("xin", 1)        # 16 tags x 4KB (64KB/p)
        res2 = pool("res2", 1)      # 16 tags x 4KB + 2 ln tags (72KB/p)
        wp = pool("wp", 1)          # 16 tags x 1KB (16KB/p)
        stp = pool("stp", 5)        # 1 tag x 5 x 4KB (20KB/p)
        bcp = pool("bcp", 2)        # 1 tag x 2 x 4KB (8KB/p)
        tmp = pool("tmp", 3)        # 1 tag x 3 x 4KB (12KB/p)
        evp = pool("evp", 2)        # 1 tag x 2 x 2KB (4KB/p)
        a0p = pool("a0p", 3)        # 1 tag x 3 x 2KB (6KB/p)
        cst = pool("cst", 1)        # constants (~1.5KB/p)
        psp = tc.tile_pool(name="psp", bufs=1, space="PSUM")
        ctxs.append(psp)
        psp = psp.__enter__()

        # constants
        bias_sb = cst.tile([128, 21, NT], f32)
        nc.sync.dma_start(bias_sb[:], bpk.rearrange("s p f -> p s f"))
        ones_r = cst.tile([128, 1], f32r)
        nc.sync.dma_start(ones_r[:], onesc[:, 0:1])
        ones1_f = cst.tile([1, 128], f32)
        nc.vector.memset(ones1_f[:], 1.0)
        ones1_r = cst.tile([1, 128], f32r)
        nc.sync.dma_start(ones1_r[:], onesc[0:1, :])
        eps_t = cst.tile([1, 1], f32)
        nc.vector.memset(eps_t[:], EPS)

        # residual accumulator starts as views[0] (feature-major); the three
        # Wout partial products are DMA-accumulated into it during P5
        nc.sync.dma_start(xacc[:, :], xT[0])

        # the two resident 16-tile sets; phases ping-pong between them so a
        # build into one set overlaps the projection reading the other
        def rtile(which, t, dt=f32r, name=None):
            pl, tg = (xin, "x") if which == 0 else (res2, "r")
            return pl.tile([128, BC], dt, tag=f"{tg}{t}", name=name or f"{tg}t{t}")

        def load16(src2d, which):
            ts = []
            for t in range(NT):
                tl = rtile(which, t)
                nc.sync.dma_start(tl[:], src2d[t * 128:(t + 1) * 128, :])
                ts.append(tl)
            return ts

        def evict(dst_ap, ps_ap, bidx, gt):
            if bidx is None:
                nc.scalar.activation(dst_ap, ps_ap, AF.Copy)
            else:
                nc.scalar.activation(dst_ap, ps_ap, AF.Identity,
                                     bias=bias_sb[:, bidx, gt:gt + 1])

        def proj(w2d, xt, dst, bidx=None, dst_sb=None, also_dst=None,
                 accum=False):
            """dst[g,b] = sum_h w2d[h,g] x[h,b] (+bias[g]).

            2-g-tile PSUM groups on banks pp0-3 only (pp4-7 stay free for
            the attention/LN small matmuls).  dst_sb: write into SBUF tiles;
            also_dst: additionally DMA dst_sb out to DRAM.
            """
            for gg in range(8):
                wb = []
                for ht in range(NT):
                    w = wp.tile([128, 256], f32r, tag=f"w{ht}", name=f"w{ht}")
                    nc.sync.dma_start(
                        w[:], w2d[ht * 128:(ht + 1) * 128,
                                  gg * 256:(gg + 1) * 256])
                    wb.append(w)
                for hf in range(2):
                    sl = slice(hf * HALF, (hf + 1) * HALF)
                    pts = [psp.tile([128, 512], f32, tag=f"pp{gi + 2 * hf}",
                                    name=f"pt{gi}") for gi in range(2)]
                    for ht in range(NT):
                        for gi in range(2):
                            nc.tensor.matmul(
                                pts[gi][:],
                                wb[ht][:, gi * 128:(gi + 1) * 128],
                                xt[ht][:, sl],
                                start=(ht == 0), stop=(ht == NT - 1))
                    for gi in range(2):
                        gt = gg * 2 + gi
                        if dst_sb is not None:
                            evict(dst_sb[gt][:, sl], pts[gi][:], bidx, gt)
                            if also_dst is not None:
                                nc.sync.dma_start(
                                    also_dst[gt * 128:(gt + 1) * 128, sl],
                                    dst_sb[gt][:, sl])
                        else:
                            et = evp.tile([128, 512], f32r, tag="ev",
                                          name="ev")
                            evict(et[:], pts[gi][:], bidx, gt)
                            if accum:
                                nc.gpsimd.dma_start(
                                    dst[gt * 128:(gt + 1) * 128, sl], et[:],
                                    accum_op=ALU.add)
                            else:
                                nc.sync.dma_start(
                                    dst[gt * 128:(gt + 1) * 128, sl], et[:])

        # ===== P1: per view: k, v (kept + spilled), v21s, q, q2 =====
        for v in range(V):
            xt = load16(xT[v], 0)
            proj(wk[v], xt, k_s[v], bidx=3 + v)
            if v == 0:
                proj(wv[v], xt, v_s[v], bidx=6 + v)
            else:
                vres = [rtile(1, t, name=f"vres{t}") for t in range(NT)]
                proj(wv[v], xt, None, bidx=6 + v, dst_sb=vres,
                     also_dst=v_s[v])
                for i in range(V):
                    if S1[i] == v:
                        proj(wiv[i], vres, v21_s[i], bidx=12 + i)
            qres = [rtile(1, t, name=f"qres{t}") for t in range(NT)]
            proj(wq[v], xt, None, bidx=0 + v, dst_sb=qres)
            proj(wiq[v], qres, q2_s[v], bidx=9 + v)

        # ===== P3: dk2 = Wik @ (k[s0]-k[s1]); sets ping-pong 0,1,0 =====
        for i in range(V):
            which = i % 2
            kd = []
            for t in range(NT):
                k0 = stp.tile([128, BC], f32r, tag="st", name="k0")
                k1 = stp.tile([128, BC], f32r, tag="st", name="k1")
                nc.sync.dma_start(k0[:], k_s[S0[i]][t * 128:(t + 1) * 128, :])
                nc.sync.dma_start(k1[:], k_s[S1[i]][t * 128:(t + 1) * 128, :])
                kt = rtile(which, t, name=f"kd{t}")
                eng = nc.vector if t % 2 == 0 else nc.gpsimd
                eng.tensor_tensor(kt[:], k0[:], k1[:], ALU.subtract)
                kd.append(kt)
            proj(wik[i], kd, dk2_s[i])
            # attention scores: a0 = sigmoid(colsum(q2*dk2)/sqrt(HD)) -> DRAM.
            # Uses only PSUM banks pp4-7, so it fills PE bubbles without
            # contending with the projection pipeline on pp0-3.
            for t in range(NT):
                q2t = stp.tile([128, BC], f32r, tag="st", name="q2t")
                dkt = stp.tile([128, BC], f32r, tag="st", name="dkt")
                nc.sync.dma_start(q2t[:], q2_s[i][t * 128:(t + 1) * 128, :])
                nc.sync.dma_start(dkt[:], dk2_s[i][t * 128:(t + 1) * 128, :])
                pt = tmp.tile([128, BC], f32r, tag="tm", name="pt")
                en2 = nc.vector if t % 2 == 0 else nc.gpsimd
                en2.tensor_tensor(pt[:], q2t[:], dkt[:], ALU.mult)
                for hf in range(2):
                    sl = slice(hf * HALF, (hf + 1) * HALF)
                    cs = psp.tile([128, 512], f32,
                                  tag=f"pp{4 + (2 * t + hf) % 4}", name="cs")
                    nc.tensor.matmul(cs[0:1, :], ones_r[:], pt[:, sl],
                                     start=True, stop=True)
                    a0t = a0p.tile([1, 512], f32r, tag="a0", name="a0t")
                    nc.scalar.activation(a0t[:], cs[0:1, :], AF.Sigmoid,
                                         scale=SCALE)
                    nc.sync.dma_start(a0_d[i, t:t + 1, sl], a0t[:])

        # ===== P4: dv2 = Wiv @ (v[s0]-v[s1]); sets ping-pong 1,0,1 =====
        for i in range(V):
            which = (i + 1) % 2
            vd = []
            for t in range(NT):
                v0 = stp.tile([128, BC], f32r, tag="st", name="v0")
                v1 = stp.tile([128, BC], f32r, tag="st", name="v1")
                nc.sync.dma_start(v0[:], v_s[S0[i]][t * 128:(t + 1) * 128, :])
                nc.sync.dma_start(v1[:], v_s[S1[i]][t * 128:(t + 1) * 128, :])
                vdt = rtile(which, t, name=f"vd{t}")
                eng = nc.vector if t % 2 == 0 else nc.gpsimd
                eng.tensor_tensor(vdt[:], v0[:], v1[:], ALU.subtract)
                vd.append(vdt)
            proj(wiv[i], vd, dv2_s[i])

        # ===== P5: attention (A) + Wo (B) + Wout (C), interleaved =====
        # ctx -> set 0 (xin), att -> set 1 (res2).  A(i) hides under C(i-1);
        # colsums use dedicated PSUM banks pp6/pp7; a0 broadcast via DMA.
        def attn_ctx(i):
            # ctx = v21 + a0*(dv2): no PE/PSUM usage at all -- a0 comes back
            # from DRAM through a partition-broadcast DMA, elementwise work
            # is split halves across DVE and GpSimd.
            ctx_t = []
            h0 = slice(0, HALF)
            h1 = slice(HALF, BC)
            for t in range(NT):
                dvt = stp.tile([128, BC], f32r, tag="st", name="dvt")
                v1t = stp.tile([128, BC], f32r, tag="st", name="v1t")
                nc.sync.dma_start(dvt[:], dv2_s[i][t * 128:(t + 1) * 128, :])
                nc.sync.dma_start(v1t[:], v21_s[i][t * 128:(t + 1) * 128, :])
                bct = bcp.tile([128, BC], f32r, tag="bc", name="bct")
                src = a0_d[i, t]
                a0b = bass.AP(tensor=src.tensor, offset=src.offset,
                              ap=[[0, 128], [1, BC]])
                nc.sync.dma_start(bct[:], a0b)
                t2 = tmp.tile([128, BC], f32r, tag="tm", name="t2")
                ct = rtile(0, t, name=f"ctx{t}")
                nc.vector.tensor_tensor(t2[:, h0], dvt[:, h0], bct[:, h0],
                                        ALU.mult)
                nc.vector.tensor_tensor(ct[:, h0], t2[:, h0], v1t[:, h0],
                                        ALU.add)
                nc.gpsimd.tensor_tensor(t2[:, h1], dvt[:, h1], bct[:, h1],
                                        ALU.mult)
                nc.gpsimd.tensor_tensor(ct[:, h1], t2[:, h1], v1t[:, h1],
                                        ALU.add)
                ctx_t.append(ct)
            return ctx_t

        def proj_B(i, ctx_t):
            att = [rtile(1, t, name=f"att{t}") for t in range(NT)]
            proj(wo[i], ctx_t, None, bidx=15 + i, dst_sb=att)
            return att

        def proj_C(i, att):
            proj(wout[i], att, xacc, bidx=(18 if i == 0 else None),
                 accum=True)

        att_prev = proj_B(0, attn_ctx(0))
        for i in range(1, V):
            proj_C(i - 1, att_prev)
            att_prev = proj_B(i, attn_ctx(i))
        proj_C(V - 1, att_prev)

        # ===== P6: residual + LayerNorm (feature-dim stats via PE) =====
        xln = []
        sx = [psp.tile([128, 512], f32, tag="pp4", name="sx0"),
              psp.tile([128, 512], f32, tag="pp5", name="sx1")]
        sxx = [psp.tile([128, 512], f32, tag="pp6", name="sxx0"),
               psp.tile([128, 512], f32, tag="pp7", name="sxx1")]
        for t in range(NT):
            eng = nc.vector if t % 2 == 0 else nc.gpsimd
            xt = rtile(0, t, name=f"xln{t}")
            nc.sync.dma_start(xt[:], xacc[t * 128:(t + 1) * 128, :])
            sq = tmp.tile([128, BC], f32r, tag="tm", name="sq")
            eng.tensor_tensor(sq[:], xt[:], xt[:], ALU.mult)
            for hf in range(2):
                sl = slice(hf * HALF, (hf + 1) * HALF)
                nc.tensor.matmul(sx[hf][0:1, :], ones_r[:], xt[:, sl],
                                 start=(t == 0), stop=(t == NT - 1))
                nc.tensor.matmul(sxx[hf][0:1, :], ones_r[:], sq[:, sl],
                                 start=(t == 0), stop=(t == NT - 1))
            xln.append(xt)
        mu = res2.tile([1, BC], f32, tag="ln0", name="mu")
        m2 = res2.tile([1, BC], f32, tag="ln1", name="m2")
        for hf in range(2):
            sl = slice(hf * HALF, (hf + 1) * HALF)
            nc.scalar.activation(mu[:, sl], sx[hf][0:1, :], AF.Copy,
                                 scale=1.0 / H)
            nc.scalar.activation(m2[:, sl], sxx[hf][0:1, :], AF.Copy,
                                 scale=1.0 / H)
            msq = a0p.tile([1, 512], f32, tag="a0", name="msq")
            nc.vector.tensor_tensor(msq[:], mu[:, sl], mu[:, sl], ALU.mult)
            nc.vector.tensor_tensor(m2[:, sl], m2[:, sl], msq[:],
                                    ALU.subtract)
        nc.scalar.activation(m2[:], m2[:], AF.Sqrt, bias=eps_t[:])
        nc.vector.reciprocal(m2[:], m2[:])          # rstd
        nc.vector.tensor_tensor(mu[:], mu[:], m2[:], ALU.mult)
        nc.scalar.activation(mu[:], mu[:], AF.Copy, scale=-1.0)  # -mu*rstd
        A_sb = rtile(1, 0, dt=f32, name="Asb")
        B_sb = rtile(1, 1, dt=f32, name="Bsb")
        for hf in range(2):
            sl = slice(hf * HALF, (hf + 1) * HALF)
            pa = psp.tile([128, 512], f32, tag="pp0", name="pa")
            nc.tensor.matmul(pa[:], ones1_f[:], m2[:, sl], start=True,
                             stop=True)
            nc.scalar.activation(A_sb[:, sl], pa[:], AF.Copy)
            pb = psp.tile([128, 512], f32, tag="pp1", name="pb")
            nc.tensor.matmul(pb[:], ones1_f[:], mu[:, sl], start=True,
                             stop=True)
            nc.scalar.activation(B_sb[:, sl], pb[:], AF.Copy)
        for t in range(NT):
            eng = nc.vector if t % 2 == 0 else nc.gpsimd
            n1 = tmp.tile([128, BC], f32, tag="tm", name="n1")
            eng.tensor_tensor(n1[:], xln[t][:].bitcast(f32), A_sb[:],
                              ALU.mult)
            eng.tensor_tensor(n1[:], n1[:], B_sb[:], ALU.add)
            eng.tensor_scalar(
                out=n1[:], in0=n1[:],
                scalar1=bias_sb[:, 19, t:t + 1],
                scalar2=bias_sb[:, 20, t:t + 1],
                op0=ALU.mult, op1=ALU.add)
            nc.sync.dma_start(out[t * 128:(t + 1) * 128, :], n1[:])

        for p in reversed(ctxs):
            p.__exit__(None, None, None)

    nc.compile()
    return nc


def _prep_host(inputs):
    """Transpose/pack host inputs (layout only, no math)."""
    views = np.asarray(inputs["views"], np.float32)

    def t3(a):
        return np.ascontiguousarray(np.asarray(a, np.float32).transpose(0, 2, 1))

    w = {
        "wq": t3(inputs["Wq"]), "wk": t3(inputs["Wk"]), "wv": t3(inputs["Wv"]),
        "wiq": t3(inputs["Wiq"]), "wik": t3(inputs["Wik"]),
        "wiv": t3(inputs["Wiv"]), "wo": t3(inputs["Wo"]),
        "wout": np.ascontiguousarray(
            np.asarray(inputs["Wout"], np.float32).T.reshape(V, H, H)),
    }

    def bcol(vec):
        return np.asarray(vec, np.float32).reshape(NT, 128).T

    bp = np.zeros((21, 128, NT), np.float32)
    for v in range(V):
        bp[0 + v] = bcol(inputs["bq"][v])
        bp[3 + v] = bcol(inputs["bk"][v])
        bp[6 + v] = bcol(inputs["bv"][v])
        bp[9 + v] = bcol(inputs["biq"][v])
        bp[12 + v] = bcol(inputs["biv"][v])
        bp[15 + v] = bcol(inputs["bo"][v])
    bp[18] = bcol(inputs["bout"])
    bp[19] = bcol(inputs["gamma"])
    bp[20] = bcol(inputs["beta"])
    w["bpk"] = bp
    w["onesc"] = np.ones((128, 128), np.float32)

    xts = []
    for c in range(N_CORES):
        sl = views[:, c * BC:(c + 1) * BC, :]
        xts.append(np.ascontiguousarray(sl.transpose(0, 2, 1)))
    return w, xts


def kernel(**inputs):
    from concourse.bass_utils import run_bass_kernel_spmd

    trace = bool(_CACHE.get("trace", False))
    if "nc" not in _CACHE:
        _CACHE["nc"] = _build_program()
    nc = _CACHE["nc"]

    w, xts = _prep_host(inputs)
    in_maps = []
    for c in range(N_CORES):
        m = dict(w)
        m["xT"] = xts[c]
        in_maps.append(m)

    res = run_bass_kernel_spmd(nc, in_maps, core_ids=list(range(N_CORES)),
                               trace=trace)
    _CACHE["last_result"] = res

    outp = np.empty((B, H), np.float32)
    for c in range(N_CORES):
        outp[c * BC:(c + 1) * BC, :] = res.results[c]["out"].T
    return outp



# revision 9
# speedup vs baseline: 1.6238x; 1.6238x over previous
"""Trainium2 Bass kernel for nn_CrossAttentionFusion (V=3, B=8192, H=2048, NH=16).

Strategy (v2 — restructured):
  - Data-parallel: batch B=8192 split across 8 NeuronCores (Bc=1024 each).
  - Feature-major activations on device: every tensor is [H, Bc] so all
    projections are PE matmuls with no on-device transposes.
  - Host-side weight fusion removes chained projections:
        q2  = (Wiq Wq) x_i                            (WQ2)
        dk2 = (Wik Wk[s0]) x_s0 - (Wik Wk[s1]) x_s1   (KA, KB; bik cancels)
        va0 = (Wiv Wv[s0]) x_s0, vb1 = (Wiv Wv[s1]) x_s1
        y   = sum_i (Wout_i Wo_i) ctx_i               (WoC)
    27 HxH matmuls/core -> 18.  All matmuls bf16 (same PE rate as fp32r,
    half the DMA + SBUF), accumulated in fp32 PSUM.
  - Softmax over V-1=2 key views collapses to a sigmoid:
        a0 = sigmoid((q2 . dk2)/sqrt(HD)) per head (head == 128-row tile)
        ctx = vb1 + a0*(va0 - vb1)
  - Everything SBUF-resident: x (3 views, bf16) and ctx tiles stay on chip;
    only a0 (tiny) round-trips DRAM for the partition-broadcast, and the
    final y accumulates into DRAM xacc.
"""

import math

import numpy as np

V = 3
B = 8192
H = 2048
NH = 16
HD = H // NH
EPS = 1e-5
N_CORES = 8
BC = B // N_CORES          # 1024 batch columns per core
NT = H // 128              # 16 h-tiles (== NH heads, HD == 128)
HALF = 512                 # matmul moving free dim
SCALE = 1.0 / math.sqrt(HD)

# others[i] = sources of keys/values for query view i
S0 = [1, 0, 0]
S1 = [2, 2, 1]

# bias-pack rows
BQ, BK, BV0, BV1, BOUT, GAM, BET = 0, 3, 6, 9, 12, 13, 14
NB = 15

_CACHE = {}


def _build_program():
    import concourse.bass as bass
    import concourse.bacc as bacc
    import concourse.tile as tile
    import concourse.mybir as mybir

    f32 = mybir.dt.float32
    f32r = mybir.dt.float32r
    bf16 = mybir.dt.bfloat16
    AF = mybir.ActivationFunctionType
    ALU = mybir.AluOpType

    nc = bacc.Bacc("TRN2", target_bir_lowering=False, debug=False,
                   num_devices=N_CORES)

    # ---- External I/O ----
    xT = nc.dram_tensor("xT", [V, H, BC], bf16, kind="ExternalInput").ap()
    wq2 = nc.dram_tensor("wq2", [V, H, H], bf16, kind="ExternalInput").ap()
    wka = nc.dram_tensor("wka", [V, H, H], bf16, kind="ExternalInput").ap()
    wkb = nc.dram_tensor("wkb", [V, H, H], bf16, kind="ExternalInput").ap()
    wva = nc.dram_tensor("wva", [V, H, H], bf16, kind="ExternalInput").ap()
    wvb = nc.dram_tensor("wvb", [V, H, H], bf16, kind="ExternalInput").ap()
    woc = nc.dram_tensor("woc", [V, H, H], bf16, kind="ExternalInput").ap()
    bpk = nc.dram_tensor("bpk", [NB, 128, NT], f32, kind="ExternalInput").ap()
    onesd = nc.dram_tensor("onesd", [128, 1], f32r, kind="ExternalInput").ap()
    out = nc.dram_tensor("out", [H, BC], f32, kind="ExternalOutput").ap()

    # ---- DRAM scratch ----
    xacc = nc.dram_tensor("xacc", [H, BC], f32r).ap()
    a0_d = nc.dram_tensor("a0_d", [V, NT, BC], bf16).ap()
    ab_d = nc.dram_tensor("ab_d", [2, BC], f32).ap()

    with tile.TileContext(nc) as tc:
        ctxs = []

        def pool(name, bufs, space=None):
            kw = dict(name=name, bufs=bufs)
            if space:
                kw["space"] = space
            p = tc.tile_pool(**kw)
            ctxs.append(p)
            return p.__enter__()

        xin = pool("xin", 1)       # 48 tags x 2KB  (96KB/p)
        ctxp = pool("ctxp", 1)     # 16 tags x 2KB  (32KB/p)
        wp = pool("wp", 1)         # 48 tags x 512B (24KB/p)
        scp = pool("scp", 1)       # q2/dk/pr tags  (12KB/p)
        bcp = pool("bcp", 2)       # bc tag x2      (4KB/p)
        evp = pool("evp", 2)       # ev tag x2      (4KB/p)
        a0p = pool("a0p", 2)       # a0/msq tag x2  (4KB/p)
        lnpa = pool("lnpa", 1)     # ln/nf         (8KB/p)
        lnpb = pool("lnpb", 1)     # sq            (2KB/p)
        lns = pool("lns", 1)       # A/B/mu/m2     (16KB/p)
        cst = pool("cst", 1)       # constants     (~2KB/p)
        psp = pool("psp", 1, space="PSUM")

        # constants
        bias_sb = cst.tile([128, NB, NT], f32)
        nc.sync.dma_start(bias_sb[:], bpk.rearrange("s p f -> p s f"))
        ones_bf = cst.tile([128, 1], bf16)
        nc.vector.memset(ones_bf[:], 1.0)
        ones_r = cst.tile([128, 1], f32r)
        nc.sync.dma_start(ones_r[:], onesd)
        eps_t = cst.tile([1, 1], f32)
        nc.vector.memset(eps_t[:], EPS)

        # resident x tiles: 3 views x 16 h-tiles, bf16
        xs = []
        for v in range(V):
            ts = []
            for t in range(NT):
                tl = xin.tile([128, BC], bf16, tag=f"x{v}_{t}",
                              name=f"x{v}_{t}")
                nc.sync.dma_start(tl[:], xT[v][t * 128:(t + 1) * 128, :])
                ts.append(tl)
            xs.append(ts)

        def ldw(w3, i, gg, tagpfx):
            """Load the 16 h-tiles of weight columns [gg*256,(gg+1)*256)."""
            ws = []
            for ht in range(NT):
                w = wp.tile([128, 256], bf16, tag=f"{tagpfx}{ht}",
                            name=f"{tagpfx}{ht}")
                nc.sync.dma_start(
                    w[:], w3[i][ht * 128:(ht + 1) * 128,
                                gg * 256:(gg + 1) * 256])
                ws.append(w)
            return ws

        for i in range(V):
            s0, s1 = S0[i], S1[i]

            # ===== SCORE phase: q2, dk2, a0 =====
            for gg in range(8):
                wq_t = ldw(wq2, i, gg, "wq")
                wa_t = ldw(wka, i, gg, "wa")
                wb_t = ldw(wkb, i, gg, "wb")
                for gi in range(2):
                    gt = gg * 2 + gi
                    q2t = scp.tile([128, BC], bf16, tag=f"q2_{gi}",
                                   name=f"q2_{gt}")
                    dkt = scp.tile([128, BC], bf16, tag=f"dk_{gi}",
                                   name=f"dk_{gt}")
                    for bh in range(2):
                        sl = slice(bh * HALF, (bh + 1) * HALF)
                        pq = psp.tile([128, HALF], f32, tag=f"p{gi}",
                                      name="pq")
                        for ht in range(NT):
                            nc.tensor.matmul(
                                pq[:], wq_t[ht][:, gi * 128:(gi + 1) * 128],
                                xs[i][ht][:, sl],
                                start=(ht == 0), stop=(ht == NT - 1))
                        pk = psp.tile([128, HALF], f32, tag=f"p{2 + gi}",
                                      name="pk")
                        for ht in range(NT):
                            nc.tensor.matmul(
                                pk[:], wa_t[ht][:, gi * 128:(gi + 1) * 128],
                                xs[s0][ht][:, sl],
                                start=(ht == 0), stop=False)
                        for ht in range(NT):
                            nc.tensor.matmul(
                                pk[:], wb_t[ht][:, gi * 128:(gi + 1) * 128],
                                xs[s1][ht][:, sl],
                                start=False, stop=(ht == NT - 1))
                        nc.scalar.activation(
                            q2t[:, sl], pq[:], AF.Identity,
                            bias=bias_sb[:, BQ + i, gt:gt + 1])
                        nc.scalar.activation(
                            dkt[:, sl], pk[:], AF.Identity,
                            bias=bias_sb[:, BK + i, gt:gt + 1])
                    pr = scp.tile([128, BC], bf16, tag=f"pr{gi}",
                                  name=f"pr{gt}")
                    nc.vector.tensor_tensor(pr[:, 0:HALF], q2t[:, 0:HALF],
                                            dkt[:, 0:HALF], ALU.mult)
                    nc.gpsimd.tensor_tensor(pr[:, HALF:BC], q2t[:, HALF:BC],
                                            dkt[:, HALF:BC], ALU.mult)
                    for bh in range(2):
                        sl = slice(bh * HALF, (bh + 1) * HALF)
                        cs = psp.tile([128, HALF], f32,
                                      tag=f"p{4 + (2 * gi + bh) % 4}",
                                      name="cs")
                        nc.tensor.matmul(cs[0:1, :], ones_bf[:], pr[:, sl],
                                         start=True, stop=True)
                        a0t = a0p.tile([1, HALF], bf16, tag="a0", name="a0t")
                        nc.scalar.activation(a0t[:], cs[0:1, :], AF.Sigmoid,
                                             scale=SCALE)
                        nc.sync.dma_start(a0_d[i, gt:gt + 1, sl], a0t[:])

            # ===== VALUE phase: va0, vb1, ctx =====
            ctx_t = [None] * NT
            for gg in range(8):
                wa_t = ldw(wva, i, gg, "wa")
                wb_t = ldw(wvb, i, gg, "wb")
                for gi in range(2):
                    gt = gg * 2 + gi
                    vat = scp.tile([128, BC], bf16, tag=f"q2_{gi}",
                                   name=f"va_{gt}")
                    vbt = scp.tile([128, BC], bf16, tag=f"dk_{gi}",
                                   name=f"vb_{gt}")
                    for bh in range(2):
                        sl = slice(bh * HALF, (bh + 1) * HALF)
                        pva = psp.tile([128, HALF], f32, tag=f"p{gi}",
                                       name="pva")
                        for ht in range(NT):
                            nc.tensor.matmul(
                                pva[:], wa_t[ht][:, gi * 128:(gi + 1) * 128],
                                xs[s0][ht][:, sl],
                                start=(ht == 0), stop=(ht == NT - 1))
                        pvb = psp.tile([128, HALF], f32, tag=f"p{2 + gi}",
                                       name="pvb")
                        for ht in range(NT):
                            nc.tensor.matmul(
                                pvb[:], wb_t[ht][:, gi * 128:(gi + 1) * 128],
                                xs[s1][ht][:, sl],
                                start=(ht == 0), stop=(ht == NT - 1))
                        nc.scalar.activation(
                            vat[:, sl], pva[:], AF.Identity,
                            bias=bias_sb[:, BV0 + i, gt:gt + 1])
                        nc.scalar.activation(
                            vbt[:, sl], pvb[:], AF.Identity,
                            bias=bias_sb[:, BV1 + i, gt:gt + 1])
                    # ctx = vb1 + a0*(va0 - vb1); a0 broadcast over partitions
                    bct = bcp.tile([128, BC], bf16, tag="bc", name="bct")
                    src = a0_d[i, gt]
                    a0b = bass.AP(tensor=src.tensor, offset=src.offset,
                                  ap=[[0, 128], [1, BC]])
                    nc.sync.dma_start(bct[:], a0b)
                    dv = scp.tile([128, BC], bf16, tag=f"pr{gi}",
                                  name=f"dv{gt}")
                    ct = ctxp.tile([128, BC], bf16, tag=f"c{gt}",
                                   name=f"ctx{gt}")
                    h0 = slice(0, HALF)
                    h1 = slice(HALF, BC)
                    nc.vector.tensor_tensor(dv[:, h0], vat[:, h0], vbt[:, h0],
                                            ALU.subtract)
                    nc.gpsimd.tensor_tensor(dv[:, h1], vat[:, h1], vbt[:, h1],
                                            ALU.subtract)
                    nc.vector.tensor_tensor(dv[:, h0], dv[:, h0], bct[:, h0],
                                            ALU.mult)
                    nc.gpsimd.tensor_tensor(dv[:, h1], dv[:, h1], bct[:, h1],
                                            ALU.mult)
                    nc.vector.tensor_tensor(ct[:, h0], dv[:, h0], vbt[:, h0],
                                            ALU.add)
                    nc.gpsimd.tensor_tensor(ct[:, h1], dv[:, h1], vbt[:, h1],
                                            ALU.add)
                    ctx_t[gt] = ct

            # ===== OUTPUT phase: y += WoC ctx =====
            for gg in range(8):
                wo_t = ldw(woc, i, gg, "wq")
                for gi in range(2):
                    gt = gg * 2 + gi
                    for bh in range(2):
                        sl = slice(bh * HALF, (bh + 1) * HALF)
                        py = psp.tile([128, HALF], f32,
                                      tag=f"p{(2 * gi + bh) % 4}", name="py")
                        for ht in range(NT):
                            nc.tensor.matmul(
                                py[:], wo_t[ht][:, gi * 128:(gi + 1) * 128],
                                ctx_t[ht][:, sl],
                                start=(ht == 0), stop=(ht == NT - 1))
                        et = evp.tile([128, HALF], f32r, tag="ev", name="ev")
                        if i == 0:
                            nc.scalar.activation(
                                et[:], py[:], AF.Identity,
                                bias=bias_sb[:, BOUT, gt:gt + 1])
                            nc.sync.dma_start(
                                xacc[gt * 128:(gt + 1) * 128, sl], et[:])
                        else:
                            nc.scalar.activation(et[:], py[:], AF.Copy)
                            nc.gpsimd.dma_start(
                                xacc[gt * 128:(gt + 1) * 128, sl], et[:],
                                accum_op=ALU.add)

        # ===== LN phase: x = x0 + xacc; out = LN(x)*gamma+beta =====
        # pass 1: stats (colsum of x and x^2 over feature dim via PE)
        sx = [psp.tile([128, HALF], f32, tag="p4", name="sx0"),
              psp.tile([128, HALF], f32, tag="p5", name="sx1")]
        sxx = [psp.tile([128, HALF], f32, tag="p6", name="sxx0"),
               psp.tile([128, HALF], f32, tag="p7", name="sxx1")]

        def make_x(t):
            xat = lnpa.tile([128, BC], f32r, tag="ln", name=f"xa{t}")
            nc.sync.dma_start(xat[:], xacc[t * 128:(t + 1) * 128, :])
            nf = lnpa.tile([128, BC], f32r, tag="nf", name=f"nf{t}")
            nc.scalar.activation(nf[:], xs[0][t][:], AF.Copy)
            eng = nc.vector if t % 2 == 0 else nc.gpsimd
            eng.tensor_tensor(nf[:], nf[:], xat[:], ALU.add)
            return nf, eng

        for t in range(NT):
            nf, eng = make_x(t)
            sq = lnpb.tile([128, BC], bf16, tag="sq", name=f"sq{t}")
            eng.tensor_tensor(sq[:], nf[:], nf[:], ALU.mult)
            for hf in range(2):
                sl = slice(hf * HALF, (hf + 1) * HALF)
                nc.tensor.matmul(sx[hf][0:1, :], ones_r[:], nf[:, sl],
                                 start=(t == 0), stop=(t == NT - 1))
                nc.tensor.matmul(sxx[hf][0:1, :], ones_bf[:], sq[:, sl],
                                 start=(t == 0), stop=(t == NT - 1))
        mu = lns.tile([1, BC], f32, tag="mu", name="mu")
        m2 = lns.tile([1, BC], f32, tag="m2", name="m2")
        for hf in range(2):
            sl = slice(hf * HALF, (hf + 1) * HALF)
            nc.scalar.activation(mu[:, sl], sx[hf][0:1, :], AF.Copy,
                                 scale=1.0 / H)
            nc.scalar.activation(m2[:, sl], sxx[hf][0:1, :], AF.Copy,
                                 scale=1.0 / H)
            msq = a0p.tile([1, HALF], f32, tag="a0", name="msq")
            nc.vector.tensor_tensor(msq[:], mu[:, sl], mu[:, sl], ALU.mult)
            nc.vector.tensor_tensor(m2[:, sl], m2[:, sl], msq[:],
                                    ALU.subtract)
        nc.scalar.activation(m2[:], m2[:], AF.Sqrt, bias=eps_t[:])
        nc.vector.reciprocal(m2[:], m2[:])          # rstd
        nc.vector.tensor_tensor(mu[:], mu[:], m2[:], ALU.mult)
        nc.scalar.activation(mu[:], mu[:], AF.Copy, scale=-1.0)  # -mu*rstd
        nc.sync.dma_start(ab_d[0:1, :], m2[:])
        nc.sync.dma_start(ab_d[1:2, :], mu[:])
        A_sb = lns.tile([128, BC], f32, tag="A", name="Asb")
        B_sb = lns.tile([128, BC], f32, tag="B", name="Bsb")
        for r, dst in ((0, A_sb), (1, B_sb)):
            src = ab_d[r]
            bb = bass.AP(tensor=src.tensor, offset=src.offset,
                         ap=[[0, 128], [1, BC]])
            nc.sync.dma_start(dst[:], bb)
        # pass 2: normalize (recompute x = x0 + xacc per tile)
        for t in range(NT):
            nf, eng = make_x(t)
            nff = nf[:].bitcast(f32)
            eng.tensor_tensor(nff, nff, A_sb[:], ALU.mult)
            eng.tensor_tensor(nff, nff, B_sb[:], ALU.add)
            eng.tensor_scalar(
                out=nff, in0=nff,
                scalar1=bias_sb[:, GAM, t:t + 1],
                scalar2=bias_sb[:, BET, t:t + 1],
                op0=ALU.mult, op1=ALU.add)
            nc.sync.dma_start(out[t * 128:(t + 1) * 128, :], nff)

        for p in reversed(ctxs):
            p.__exit__(None, None, None)

    nc.compile()
    return nc


def _prep_host(inputs):
    """Fuse weight pairs (fp32) and pack to bf16 lhsT layout."""
    import ml_dtypes
    bfdt = ml_dtypes.bfloat16

    def f32a(x):
        return np.asarray(x, np.float32)

    views = f32a(inputs["views"])
    Wq, Wk, Wv = f32a(inputs["Wq"]), f32a(inputs["Wk"]), f32a(inputs["Wv"])
    Wiq, Wik, Wiv = f32a(inputs["Wiq"]), f32a(inputs["Wik"]), f32a(inputs["Wiv"])
    Wo, Wout = f32a(inputs["Wo"]), f32a(inputs["Wout"])
    bq, bk, bv = f32a(inputs["bq"]), f32a(inputs["bk"]), f32a(inputs["bv"])
    biq, bik, biv = f32a(inputs["biq"]), f32a(inputs["bik"]), f32a(inputs["biv"])
    bo, bout = f32a(inputs["bo"]), f32a(inputs["bout"])
    gamma, beta = f32a(inputs["gamma"]), f32a(inputs["beta"])

    def lhsT_stack(mats):
        """[V,H,H] bf16 array of W.T per view (lhsT layout [h,g])."""
        a = np.empty((V, H, H), bfdt)
        for i in range(V):
            a[i] = np.ascontiguousarray(mats[i].T).astype(bfdt)
        return a

    wq2 = lhsT_stack([Wiq[i] @ Wq[i] for i in range(V)])
    wka = lhsT_stack([Wik[i] @ Wk[S0[i]] for i in range(V)])
    wkb = lhsT_stack([-(Wik[i] @ Wk[S1[i]]) for i in range(V)])
    wva = lhsT_stack([Wiv[i] @ Wv[S0[i]] for i in range(V)])
    wvb = lhsT_stack([Wiv[i] @ Wv[S1[i]] for i in range(V)])
    Wout_i = [Wout[:, i * H:(i + 1) * H] for i in range(V)]
    woc = lhsT_stack([Wout_i[i] @ Wo[i] for i in range(V)])

    def bcol(vec):
        return np.asarray(vec, np.float32).reshape(NT, 128).T

    bp = np.zeros((NB, 128, NT), np.float32)
    btout = np.asarray(bout, np.float32).copy()
    for i in range(V):
        bp[BQ + i] = bcol(Wiq[i] @ bq[i] + biq[i])
        bp[BK + i] = bcol(Wik[i] @ (bk[S0[i]] - bk[S1[i]]))
        bp[BV0 + i] = bcol(Wiv[i] @ bv[S0[i]] + biv[i])
        bp[BV1 + i] = bcol(Wiv[i] @ bv[S1[i]] + biv[i])
        btout += Wout_i[i] @ bo[i]
    bp[BOUT] = bcol(btout)
    bp[GAM] = bcol(gamma)
    bp[BET] = bcol(beta)

    w = {"wq2": wq2, "wka": wka, "wkb": wkb, "wva": wva, "wvb": wvb,
         "woc": woc, "bpk": bp, "onesd": np.ones((128, 1), np.float32)}

    xts = []
    for c in range(N_CORES):
        sl = views[:, c * BC:(c + 1) * BC, :]
        xts.append(np.ascontiguousarray(sl.transpose(0, 2, 1)).astype(bfdt))
    return w, xts


def kernel(**inputs):
    from concourse.bass_utils import run_bass_kernel_spmd

    trace = bool(_CACHE.get("trace", False))
    if "nc" not in _CACHE:
        _CACHE["nc"] = _build_program()
    nc = _CACHE["nc"]

    w, xts = _prep_host(inputs)
    in_maps = []
    for c in range(N_CORES):
        m = dict(w)
        m["xT"] = xts[c]
        in_maps.append(m)

    res = run_bass_kernel_spmd(nc, in_maps, core_ids=list(range(N_CORES)),
                               trace=trace)
    _CACHE["last_result"] = res

    outp = np.empty((B, H), np.float32)
    for c in range(N_CORES):
        outp[c * BC:(c + 1) * BC, :] = res.results[c]["out"].T
    return outp
